# revision 21
# baseline (speedup 1.0000x reference)
"""MultiHeadedAttention Trainium2 Bass kernel.

Full inputs in, full output out. Sharding: 8 cores = 4 batches x 2 head-pairs
(data-parallel over batch, tensor-parallel over the 4 heads). Per core, all
matmuls in bf16 (fp32 PSUM accumulation):
  Q/K projections for its 2 heads      -> [128, 2048] bf16 (chan-major)
  V projection directly transposed     -> vt [m, (h, d+ones)] bf16
  per head/n-half: scoresT[m,n] = K^T Q, exp on ACT (scale=1/8), x[d+1, n]
  accumulated over m in PSUM with vt stationary (ones row gives softmax
  sums), normalize via DMA-broadcast + fast-approx reciprocal, out
  projection per n-half with both heads accumulated in PSUM.

Bias algebra (lets the device skip bk/bv entirely):
  - K bias shifts scores by a per-query constant -> softmax-invariant, drop.
  - V bias adds exactly bv to the normalized x (softmax weights sum to 1),
    so its output contribution is wm @ bv -> host adds (bm + wm @ bv).

Schedule notes:
  - The softmax exp on the Scalar engine (64 x [128,1024] ACTIVATEs ~71us)
    is the roofline; everything else must hide behind it.
  - PE runs 2 score-tiles AHEAD of the exp stream (deps force the static
    order [scores g+2][x-accums g]) so the PE never waits on an exp; a PE
    that waits each iteration can never assemble the ~3.4us of sustained
    activity the HAM clock gate needs to release 2.4 GHz.
  - All weights travel in ONE packed [128,1024] DMA; inputs are 3 big tiles
    loaded in 2 half-tensor DMAs each (DMA *issue* costs ~0.7us of engine
    queue time per dma_start, so fewer/bigger is better for the prologue).
  - px (unnormalized x + sums row) is evacuated to SBUF in one [65,1024]
    copy at block end so the single px PSUM buffer frees quickly.
  - Remaining projections / V^T blocks / the first out-projection group are
    emitted inside the attention loop so the static PE order interleaves
    them into PE slack instead of stalling the queue on input DMAs.
"""

import sys

if "/opt/trn_rl_repo" not in sys.path:
    sys.path.insert(0, "/opt/trn_rl_repo")

import numpy as np
import ml_dtypes

BF = ml_dtypes.bfloat16

B, D, N, H = 4, 256, 2048, 4
DIM = D // H  # 64
NW = 4  # 512-wide n windows
MB = 16  # 128-wide m blocks

_CACHE = {}


def _emit(ctx, tc, io):
    import concourse.bass as bass
    import concourse.mybir as mybir

    nc = tc.nc
    f32 = mybir.dt.float32
    bf16 = mybir.dt.bfloat16
    EXP = mybir.ActivationFunctionType.Exp

    from concourse.tile_rust import add_dep_helper

    def _raw(inst):
        return getattr(inst, "ins", inst)

    const = ctx.enter_context(tc.tile_pool(name="const", bufs=1))
    xin = ctx.enter_context(tc.tile_pool(name="xin", bufs=1))
    big = ctx.enter_context(tc.tile_pool(name="big", bufs=1))
    xpool = ctx.enter_context(tc.tile_pool(name="xpool", bufs=2))
    pb = ctx.enter_context(tc.tile_pool(name="probs", bufs=3))
    work = ctx.enter_context(tc.tile_pool(name="work", bufs=2))
    outp = ctx.enter_context(tc.tile_pool(name="outp", bufs=4))
    outf = ctx.enter_context(tc.tile_pool(name="outf", bufs=2))
    psA = ctx.enter_context(tc.tile_pool(name="psA", bufs=3, space="PSUM"))
    psX = ctx.enter_context(tc.tile_pool(name="psX", bufs=1, space="PSUM"))
    dpool = ctx.enter_context(tc.tile_pool(name="dpool", bufs=2, space="DRAM"))

    # ---- input loads: 3 big tiles, 2 half-tensor DMAs each, n-half A first
    xq_sb = xin.tile([128, 2, 2048], bf16, tag="xq")
    xk_sb = xin.tile([128, 2, 2048], bf16, tag="xk")
    xv_sb = xin.tile([128, 2, 2048], bf16, tag="xv")

    def load_cols(eng, t, name, c0, c1):
        src = io[name].rearrange("(c p) n -> p c n", p=128)
        s = slice(c0, c1)
        eng.dma_start(t[:, :, s], src[:, :, s])

    # weights first on the scalar ring (small), leading input chunks fine-
    # grained on the sync ring so the first projections start ASAP; the two
    # HWDGE rings run in parallel.
    wblob = const.tile([128, 1280], bf16, tag="wblob")
    nc.scalar.dma_start(wblob, io["wblob"])
    bq_sb = const.tile([128, 1], f32, tag="bq")
    nc.scalar.dma_start(bq_sb, io["bq"])

    load_cols(nc.sync, xk_sb, "xk", 0, 512)
    load_cols(nc.sync, xq_sb, "xq", 0, 512)
    load_cols(nc.scalar, xq_sb, "xq", 512, 1024)
    load_cols(nc.scalar, xv_sb, "xv", 0, 512)
    load_cols(nc.sync, xk_sb, "xk", 512, 1024)
    load_cols(nc.scalar, xv_sb, "xv", 512, 1024)
    load_cols(nc.sync, xq_sb, "xq", 1024, 2048)
    load_cols(nc.scalar, xk_sb, "xk", 1024, 2048)
    load_cols(nc.sync, xv_sb, "xv", 1024, 2048)
    wqt_v = wblob[:, 0:256].rearrange("p (c o) -> p c o", c=2)
    wkt_v = wblob[:, 256:512].rearrange("p (c o) -> p c o", c=2)
    wvt_v = wblob[:, 512:768].rearrange("p (c o) -> p c o", c=2)
    wmt = (wblob[0:64, 768:1024], wblob[0:64, 1024:1280])

    # ---- small constants ----
    wu_a = const.tile([128, 128], bf16, tag="wu_a")
    nc.gpsimd.memset(wu_a, 0.0)
    wu_b = const.tile([128, 512], bf16, tag="wu_b")
    nc.gpsimd.memset(wu_b, 0.0)
    dum = const.tile([1, 16], f32, tag="dum")
    nc.gpsimd.memset(dum, 0.0)
    ones64 = const.tile([1, 64], f32, tag="ones64")
    nc.gpsimd.memset(ones64, 1.0)
    dum_o = const.tile([1, 16], bf16, tag="dum_o")
    # dummy exp: forces the ~2.7us ACT table load during the DMA ramp
    nc.scalar.activation(dum_o, dum, EXP, scale=0.125)

    # PE warmup: HAM clock gate releases (1.2 -> 2.4 GHz) after ~3.4us of
    # sustained matmul activity; the cold 512-wide matmuls bridge the input
    # DMA ramp so the projections start near-warm.
    wu_ps = psA.tile([128, 1024], f32, tag="ps", name="wu_ps")
    for i in range(6):
        nc.tensor.matmul(wu_ps[:, 0:512], lhsT=wu_a, rhs=wu_b, start=True, stop=True)

    # ---- projections (Q with bias, K bias dropped) + V^T ----
    q_sb = big.tile([128, 2048], bf16, tag="q")
    k_sb = big.tile([128, 2048], bf16, tag="k")

    qk_store = {}  # (dst_tag, window) -> producing DVE instruction
    vt_store = {}  # mb -> producing DVE instruction

    def proj_step(xt, wt, dst, w, bias=None):
        ps = psA.tile([128, 1024], f32, tag="ps", name=f"psproj{w}")
        s = slice(w * 512, (w + 1) * 512)
        nc.tensor.matmul(ps[:, 0:512], lhsT=wt[:, 0, :], rhs=xt[:, 0, s], start=True, stop=False)
        nc.tensor.matmul(ps[:, 0:512], lhsT=wt[:, 1, :], rhs=xt[:, 1, s], start=False, stop=True)
        d = dst[:, s]
        if bias is None:
            st = nc.vector.tensor_copy(d, ps[:, 0:512])
        else:
            st = nc.vector.tensor_scalar_add(d, ps[:, 0:512], bias)
        qk_store[(id(dst), w)] = st

    vt = big.tile([128, MB, 2, 65], bf16, tag="vt")
    nc.gpsimd.memset(vt[:, :, :, 64:65], 1.0)

    def vt_step(mb):
        ms = slice(mb * 128, (mb + 1) * 128)
        ps = psA.tile([128, 1024], f32, tag="ps", name=f"psvt{mb}")
        pvt = ps[:, 0:128]
        nc.tensor.matmul(pvt, lhsT=xv_sb[:, 0, ms], rhs=wvt_v[:, 0, :], start=True, stop=False)
        nc.tensor.matmul(pvt, lhsT=xv_sb[:, 1, ms], rhs=wvt_v[:, 1, :], start=False, stop=True)
        vt_store[mb] = nc.vector.tensor_copy(
            vt[:, mb, :, 0:64], pvt.rearrange("m (h d) -> m h d", h=2)
        )

    proj_step(xk_sb, wkt_v, k_sb, 0)
    proj_step(xq_sb, wqt_v, q_sb, 0, bq_sb)
    proj_step(xq_sb, wqt_v, q_sb, 1, bq_sb)
    for mb in range(4):
        vt_step(mb)

    # ---- attention ----
    x_sb = [xpool.tile([64, 2048], bf16, tag="x", name=f"x{h}") for h in range(2)]
    sc_groups = []
    xa_groups = []
    po_state = {}

    def attn_iter(h, nh, mb, px):
        qh = q_sb[h * 64 : (h + 1) * 64, :]
        kh = k_sb[h * 64 : (h + 1) * 64, :]
        nbase = nh * 1024
        pt = pb.tile([128, 1024], bf16, tag="pt", name="pt")
        sc = psA.tile([128, 1024], f32, tag="ps", name="pssc")
        scg = []
        for s2 in range(2):
            n0 = nbase + s2 * 512
            m = nc.tensor.matmul(
                sc[:, s2 * 512 : (s2 + 1) * 512],
                lhsT=kh[:, mb * 128 : (mb + 1) * 128],
                rhs=qh[:, n0 : n0 + 512],
                start=True,
                stop=True,
            )
            # explicit data deps on the q/k window stores: the region tracker
            # misses some strided producer->consumer edges (HW race).
            add_dep_helper(_raw(m), _raw(qk_store[(id(q_sb), nh * 2 + s2)]), True,
                           "scores after q window store")
            add_dep_helper(_raw(m), _raw(qk_store[(id(k_sb), mb // 4)]), True,
                           "scores after k window store")
            scg.append(m)
        nc.scalar.activation(pt, sc, EXP, scale=0.125)
        sc_groups.append(scg)
        xag = []
        for j in range(2):
            m = nc.tensor.matmul(
                px[:, j * 512 : (j + 1) * 512],
                lhsT=vt[:, mb, h, :],
                rhs=pt[:, j * 512 : (j + 1) * 512],
                start=(mb == 0),
                stop=(mb == MB - 1),
                skip_group_check=True,
            )
            add_dep_helper(_raw(m), _raw(vt_store[mb]), True,
                           "x-accum after vt block store")
            xag.append(m)
        xa_groups.append(xag)

    def normalize(h, nh, px, last=False):
        # evacuate px: x rows and the sums row (cross-partition PSUM->SBUF
        # copy), broadcast the sums row to 64 partitions on GpSimd, fast-
        # approx reciprocal, one fused multiply into the bf16 x tile.
        nbase = nh * 1024
        xe = work.tile([64, 1024], f32, tag="xe", name=f"xe{h}_{nh}")
        nc.vector.tensor_copy(xe, px[0:64, :])
        s_row = work.tile([1, 1024], f32, tag="s_row", name=f"s_row{h}_{nh}")
        nc.vector.tensor_copy(s_row, px[64:65, :])
        s_bc = work.tile([64, 1024], f32, tag="s_bc", name=f"s_bc{h}_{nh}")
        nc.gpsimd.partition_broadcast(s_bc, s_row)
        r_bc = work.tile([64, 1024], f32, tag="r_bc", name=f"r_bc{h}_{nh}")
        nc.vector.reciprocal_approx_fast(r_bc, s_bc)
        nc.vector.tensor_mul(x_sb[h][:, nbase : nbase + 1024], xe, r_bc)
        if "dbg_s0" in io:
            bi = h * 2 + nh
            nc.sync.dma_start(io[f"dbg_s{bi}"], s_row)
            nc.sync.dma_start(io[f"dbg_r{bi}"], r_bc)
            nc.sync.dma_start(io[f"dbg_xe{bi}"], xe)

    # out-projection with SBUF accumulation: each (nh, oc, h) chunk is a
    # transient 2-matmul PSUM group copied/added into an SBUF tile, so chunks
    # interleave freely with attention iterations.
    po_sb = {}

    def po_part(nh, oc, h):
        po = psA.tile([128, 1024], f32, tag="ps", name=f"po{nh}_{oc}_{h}")
        nbase = nh * 1024
        for j in range(2):
            nc.tensor.matmul(
                po[:, j * 512 : (j + 1) * 512],
                lhsT=wmt[h][:, oc * 128 : (oc + 1) * 128],
                rhs=x_sb[h][:, nbase + j * 512 : nbase + (j + 1) * 512],
                start=True,
                stop=True,
            )
        if h == 0:
            ot = outp.tile([128, 1024], f32, tag="ot", name=f"ot{nh}_{oc}")
            nc.vector.tensor_copy(ot, po)
            po_sb[(nh, oc)] = ot
        else:
            fin = outf.tile([128, 1024], f32, tag="fin", name=f"fin{nh}_{oc}")
            nc.vector.tensor_add(fin, po_sb.pop((nh, oc)), po)
            nc.sync.dma_start(
                io["out"][oc * 128 : (oc + 1) * 128, nh * 1024 : (nh + 1) * 1024],
                fin,
            )

    # interleaved emission: remaining projections / V^T inside block 0 (vt[k]
    # must be emitted before iteration k+1 -- xa[k] of EVERY block reads it),
    # h0 out-projection chunks in block 2/3, nh0-h1 chunks late in block 3.
    fill = {
        (0, 4): lambda: (proj_step(xk_sb, wkt_v, k_sb, 1), vt_step(4)),
        (0, 5): lambda: (vt_step(5), vt_step(6)),
        (0, 6): lambda: (vt_step(7), proj_step(xq_sb, wqt_v, q_sb, 2, bq_sb)),
        (0, 7): lambda: (vt_step(8), proj_step(xk_sb, wkt_v, k_sb, 2)),
        (0, 8): lambda: (vt_step(9), vt_step(10)),
        (0, 9): lambda: (vt_step(11), proj_step(xq_sb, wqt_v, q_sb, 3, bq_sb)),
        (0, 10): lambda: (vt_step(12), proj_step(xk_sb, wkt_v, k_sb, 3)),
        (0, 11): lambda: (vt_step(13), vt_step(14)),
        (0, 12): lambda: (vt_step(15),),
    }

    blocks = [(0, 0), (0, 1), (1, 0), (1, 1)]
    for bi, (h, nh) in enumerate(blocks):
        px = psX.tile([65, 1024], f32, tag="px", name=f"px{h}_{nh}")
        for mb in range(MB):
            if (bi, mb) in fill:
                fill[(bi, mb)]()
            attn_iter(h, nh, mb, px)
        normalize(h, nh, px)

    if "dbg_s0" in io:
        nc.sync.dma_start(io["dbg_q"], q_sb)
        nc.sync.dma_start(io["dbg_k"], k_sb)
        nc.sync.dma_start(io["dbg_vt"], vt)
        nc.sync.dma_start(io["dbg_x0"], x_sb[0])
        nc.sync.dma_start(io["dbg_x1"], x_sb[1])

    # tail: all out-projection chunks
    for nh in range(2):
        for oc in range(2):
            po_part(nh, oc, 0)
            po_part(nh, oc, 1)

    # PE run-ahead deps: [scores g+2][x-accums g] alternation so the PE
    # never waits on an exp (and the HAM clock gate stays open).
    G = len(sc_groups)
    for g in range(G):
        if g + 2 < G:
            for m in xa_groups[g]:
                add_dep_helper(_raw(m), _raw(sc_groups[g + 2][-1]), False,
                               "x-accums after scores g+2")
        if g + 3 < G:
            for m in sc_groups[g + 3]:
                add_dep_helper(_raw(m), _raw(xa_groups[g][-1]), False,
                               "scores g+3 after x-accums g")


def _build_nc():
    key = "nc"
    if key in _CACHE:
        return _CACHE[key]
    from contextlib import ExitStack

    import concourse.mybir as mybir
    import concourse.tile as tile
    from concourse import bacc

    f32 = mybir.dt.float32
    bf16 = mybir.dt.bfloat16
    nc = bacc.Bacc("TRN2", target_bir_lowering=False, debug=False, num_devices=8)
    io = {}
    for name, shape, dt_ in (
        ("xq", [256, 2048], bf16),
        ("xk", [256, 2048], bf16),
        ("xv", [256, 2048], bf16),
        ("wblob", [128, 1280], bf16),
        ("bq", [128, 1], f32),
    ):
        io[name] = nc.dram_tensor(name, shape, dt_, kind="ExternalInput").ap()
    io["out"] = nc.dram_tensor("out", [256, 2048], f32, kind="ExternalOutput").ap()
    import os
    if os.environ.get("KDBG", "0") == "1":
        bf16_, f32_ = bf16, f32
        for bi in range(4):
            io[f"dbg_s{bi}"] = nc.dram_tensor(f"dbg_s{bi}", [1, 1024], f32_, kind="ExternalOutput").ap()
            io[f"dbg_r{bi}"] = nc.dram_tensor(f"dbg_r{bi}", [64, 1024], f32_, kind="ExternalOutput").ap()
            io[f"dbg_xe{bi}"] = nc.dram_tensor(f"dbg_xe{bi}", [64, 1024], f32_, kind="ExternalOutput").ap()
        io["dbg_q"] = nc.dram_tensor("dbg_q", [128, 2048], bf16_, kind="ExternalOutput").ap()
        io["dbg_k"] = nc.dram_tensor("dbg_k", [128, 2048], bf16_, kind="ExternalOutput").ap()
        io["dbg_vt"] = nc.dram_tensor("dbg_vt", [128, 16, 2, 65], bf16_, kind="ExternalOutput").ap()
        io["dbg_x0"] = nc.dram_tensor("dbg_x0", [64, 2048], bf16_, kind="ExternalOutput").ap()
        io["dbg_x1"] = nc.dram_tensor("dbg_x1", [64, 2048], bf16_, kind="ExternalOutput").ap()

    with tile.TileContext(nc) as tc:
        with ExitStack() as ctx:
            _emit(ctx, tc, io)
    nc.compile()
    _CACHE[key] = nc
    return nc


def make_in_maps(query, key, value, wq, bq, wk, bk, wv, bv, wm, bm):
    fb = lambda a: np.ascontiguousarray(np.asarray(a, dtype=np.float32)).astype(BF)
    f = lambda a: np.ascontiguousarray(np.asarray(a), dtype=np.float32)
    query, key, value = f(query), f(key), f(value)
    wq, wk, wv, wm = f(wq), f(wk), f(wv), f(wm)
    bq = f(bq)

    def chan_pack(w, idx):
        # [256,128] (chan, out) -> [128, 2, 128] -> [128, 256] packed
        wt = w[idx].T.reshape(2, 128, 128).transpose(1, 0, 2).reshape(128, 256)
        return wt

    in_maps = []
    for c in range(8):
        b, pair = divmod(c, 2)
        hs = (2 * pair, 2 * pair + 1)
        idx = np.array([d * H + h for h in hs for d in range(DIM)])
        blob = np.concatenate(
            [
                chan_pack(wq, idx),
                chan_pack(wk, idx),
                chan_pack(wv, idx),
                np.pad(
                    np.concatenate([wm[:, idx[:64]].T, wm[:, idx[64:]].T], axis=1),
                    ((0, 64), (0, 0)),
                ),
            ],
            axis=1,
        )
        m = {
            "xq": fb(query[b]),
            "xk": fb(key[b]),
            "xv": fb(value[b]),
            "wblob": fb(blob),
            "bq": f(bq[idx].reshape(128, 1)),
        }
        in_maps.append(m)
    return in_maps


def run(in_maps, trace=False, **kw):
    from concourse import bass_utils

    nc = _build_nc()
    return bass_utils.run_bass_kernel_spmd(
        nc, in_maps, core_ids=list(range(8)), trace=trace, **kw
    )


def gather(results, wm, bv, bm):
    wm = np.asarray(wm, dtype=np.float32)
    bv = np.asarray(bv, dtype=np.float32)
    bm = np.asarray(bm, dtype=np.float32)
    bias = bm + wm @ bv  # device skips bv; its output contribution is wm @ bv
    outs = [np.asarray(r["out"], dtype=np.float32) for r in results]
    return np.stack([outs[2 * b] + outs[2 * b + 1] + bias[:, None] for b in range(B)])


def kernel(query, key, value, wq, bq, wk, bk, wv, bv, wm, bm):
    in_maps = make_in_maps(query, key, value, wq, bq, wk, bk, wv, bv, wm, bm)
    res = run(in_maps)
    return gather(res.results, wm, bv, bm)


# revision 23
# speedup vs baseline: 1.0348x; 1.0348x over previous
"""MultiHeadedAttention Trainium2 Bass kernel.

Full inputs in, full output out. Sharding: 8 cores = 4 batches x 2 head-pairs
(data-parallel over batch, tensor-parallel over the 4 heads). Per core, all
matmuls in bf16 (fp32 PSUM accumulation):
  Q/K projections for its 2 heads      -> [128, 2048] bf16 (chan-major)
  V projection directly transposed     -> vt [m, (h, d+ones)] bf16
  per head/n-half: scoresT[m,n] = K^T Q, exp on ACT (scale=1/8), x[d+1, n]
  accumulated over m in PSUM with vt stationary (ones row gives softmax
  sums), normalize via DMA-broadcast + fast-approx reciprocal, out
  projection per n-half with both heads accumulated in PSUM.

Bias algebra (lets the device skip bk/bv entirely):
  - K bias shifts scores by a per-query constant -> softmax-invariant, drop.
  - V bias adds exactly bv to the normalized x (softmax weights sum to 1),
    so its output contribution is wm @ bv -> host adds (bm + wm @ bv).

Schedule notes:
  - The softmax exp on the Scalar engine (64 x [128,1024] ACTIVATEs ~71us)
    is the roofline; everything else must hide behind it.
  - PE runs 2 score-tiles AHEAD of the exp stream (deps force the static
    order [scores g+2][x-accums g]) so the PE never waits on an exp; a PE
    that waits each iteration can never assemble the ~3.4us of sustained
    activity the HAM clock gate needs to release 2.4 GHz.
  - All weights travel in ONE packed [128,1024] DMA; inputs are 3 big tiles
    loaded in 2 half-tensor DMAs each (DMA *issue* costs ~0.7us of engine
    queue time per dma_start, so fewer/bigger is better for the prologue).
  - px (unnormalized x + sums row) is evacuated to SBUF in one [65,1024]
    copy at block end so the single px PSUM buffer frees quickly.
  - Remaining projections / V^T blocks / the first out-projection group are
    emitted inside the attention loop so the static PE order interleaves
    them into PE slack instead of stalling the queue on input DMAs.
"""

import sys

if "/opt/trn_rl_repo" not in sys.path:
    sys.path.insert(0, "/opt/trn_rl_repo")

import numpy as np
import ml_dtypes

BF = ml_dtypes.bfloat16

B, D, N, H = 4, 256, 2048, 4
DIM = D // H  # 64
NW = 4  # 512-wide n windows
MB = 16  # 128-wide m blocks

_CACHE = {}


def _emit(ctx, tc, io):
    import concourse.bass as bass
    import concourse.mybir as mybir

    nc = tc.nc
    f32 = mybir.dt.float32
    bf16 = mybir.dt.bfloat16
    EXP = mybir.ActivationFunctionType.Exp

    from concourse.tile_rust import add_dep_helper

    def _raw(inst):
        return getattr(inst, "ins", inst)

    const = ctx.enter_context(tc.tile_pool(name="const", bufs=1))
    xin = ctx.enter_context(tc.tile_pool(name="xin", bufs=1))
    big = ctx.enter_context(tc.tile_pool(name="big", bufs=1))
    xpool = ctx.enter_context(tc.tile_pool(name="xpool", bufs=2))
    pb = ctx.enter_context(tc.tile_pool(name="probs", bufs=3))
    work = ctx.enter_context(tc.tile_pool(name="work", bufs=2))
    outp = ctx.enter_context(tc.tile_pool(name="outp", bufs=4))
    outf = ctx.enter_context(tc.tile_pool(name="outf", bufs=2))
    psA = ctx.enter_context(tc.tile_pool(name="psA", bufs=3, space="PSUM"))
    psX = ctx.enter_context(tc.tile_pool(name="psX", bufs=1, space="PSUM"))
    dpool = ctx.enter_context(tc.tile_pool(name="dpool", bufs=2, space="DRAM"))

    # ---- input loads: 3 big tiles, 2 half-tensor DMAs each, n-half A first
    xq_sb = xin.tile([128, 2, 2048], bf16, tag="xq")
    xk_sb = xin.tile([128, 2, 2048], bf16, tag="xk")
    xv_sb = xin.tile([128, 2, 2048], bf16, tag="xv")

    def load_cols(eng, t, name, c0, c1):
        src = io[name].rearrange("(c p) n -> p c n", p=128)
        s = slice(c0, c1)
        eng.dma_start(t[:, :, s], src[:, :, s])

    # weights first on the scalar ring (small), leading input chunks fine-
    # grained on the sync ring so the first projections start ASAP; the two
    # HWDGE rings run in parallel.
    wblob = const.tile([128, 1280], bf16, tag="wblob")
    nc.scalar.dma_start(wblob, io["wblob"])
    bq_sb = const.tile([128, 1], f32, tag="bq")
    nc.scalar.dma_start(bq_sb, io["bq"])

    load_cols(nc.sync, xk_sb, "xk", 0, 512)
    load_cols(nc.sync, xq_sb, "xq", 0, 512)
    load_cols(nc.scalar, xq_sb, "xq", 512, 1024)
    load_cols(nc.scalar, xv_sb, "xv", 0, 512)
    load_cols(nc.sync, xk_sb, "xk", 512, 1024)
    load_cols(nc.scalar, xv_sb, "xv", 512, 1024)
    load_cols(nc.sync, xq_sb, "xq", 1024, 2048)
    load_cols(nc.scalar, xk_sb, "xk", 1024, 2048)
    load_cols(nc.sync, xv_sb, "xv", 1024, 2048)
    wqt_v = wblob[:, 0:256].rearrange("p (c o) -> p c o", c=2)
    wkt_v = wblob[:, 256:512].rearrange("p (c o) -> p c o", c=2)
    wvt_v = wblob[:, 512:768].rearrange("p (c o) -> p c o", c=2)
    wmt = (wblob[0:64, 768:1024], wblob[0:64, 1024:1280])

    # ---- small constants ----
    wu_a = const.tile([128, 128], bf16, tag="wu_a")
    nc.gpsimd.memset(wu_a, 0.0)
    wu_b = const.tile([128, 512], bf16, tag="wu_b")
    nc.gpsimd.memset(wu_b, 0.0)
    dum = const.tile([1, 16], f32, tag="dum")
    nc.gpsimd.memset(dum, 0.0)
    ones64 = const.tile([1, 64], f32, tag="ones64")
    nc.gpsimd.memset(ones64, 1.0)
    dum_o = const.tile([1, 16], bf16, tag="dum_o")
    # dummy exp: forces the ~2.7us ACT table load during the DMA ramp
    nc.scalar.activation(dum_o, dum, EXP, scale=0.125)

    # PE warmup: HAM clock gate releases (1.2 -> 2.4 GHz) after ~3.4us of
    # sustained matmul activity; the cold 512-wide matmuls bridge the input
    # DMA ramp so the projections start near-warm.
    wu_ps = psA.tile([128, 1024], f32, tag="ps", name="wu_ps")
    for i in range(6):
        nc.tensor.matmul(wu_ps[:, 0:512], lhsT=wu_a, rhs=wu_b, start=True, stop=True)

    # ---- projections (Q with bias, K bias dropped) + V^T ----
    q_sb = big.tile([128, 2048], bf16, tag="q")
    k_sb = big.tile([128, 2048], bf16, tag="k")

    qk_store = {}  # (dst_tag, window) -> producing DVE instruction
    vt_store = {}  # mb -> producing DVE instruction

    def proj_step(xt, wt, dst, w, bias=None):
        ps = psA.tile([128, 1024], f32, tag="ps", name=f"psproj{w}")
        s = slice(w * 512, (w + 1) * 512)
        nc.tensor.matmul(ps[:, 0:512], lhsT=wt[:, 0, :], rhs=xt[:, 0, s], start=True, stop=False)
        nc.tensor.matmul(ps[:, 0:512], lhsT=wt[:, 1, :], rhs=xt[:, 1, s], start=False, stop=True)
        d = dst[:, s]
        if bias is None:
            st = nc.vector.tensor_copy(d, ps[:, 0:512])
        else:
            st = nc.vector.tensor_scalar_add(d, ps[:, 0:512], bias)
        qk_store[(id(dst), w)] = st

    vt = big.tile([128, MB, 2, 65], bf16, tag="vt")
    nc.gpsimd.memset(vt[:, :, :, 64:65], 1.0)

    def vt_step(mb):
        ms = slice(mb * 128, (mb + 1) * 128)
        ps = psA.tile([128, 1024], f32, tag="ps", name=f"psvt{mb}")
        pvt = ps[:, 0:128]
        nc.tensor.matmul(pvt, lhsT=xv_sb[:, 0, ms], rhs=wvt_v[:, 0, :], start=True, stop=False)
        nc.tensor.matmul(pvt, lhsT=xv_sb[:, 1, ms], rhs=wvt_v[:, 1, :], start=False, stop=True)
        vt_store[mb] = nc.vector.tensor_copy(
            vt[:, mb, :, 0:64], pvt.rearrange("m (h d) -> m h d", h=2)
        )

    proj_step(xk_sb, wkt_v, k_sb, 0)
    proj_step(xq_sb, wqt_v, q_sb, 0, bq_sb)
    proj_step(xq_sb, wqt_v, q_sb, 1, bq_sb)
    for mb in range(4):
        vt_step(mb)

    # ---- attention ----
    x_sb = [xpool.tile([64, 2048], bf16, tag="x", name=f"x{h}") for h in range(2)]
    sc_groups = []
    xa_groups = []
    po_state = {}

    def attn_iter(h, nh, mb, px):
        qh = q_sb[h * 64 : (h + 1) * 64, :]
        kh = k_sb[h * 64 : (h + 1) * 64, :]
        nbase = nh * 1024
        pt = pb.tile([128, 1024], bf16, tag="pt", name="pt")
        sc = psA.tile([128, 1024], f32, tag="ps", name="pssc")
        scg = []
        for s2 in range(2):
            n0 = nbase + s2 * 512
            m = nc.tensor.matmul(
                sc[:, s2 * 512 : (s2 + 1) * 512],
                lhsT=kh[:, mb * 128 : (mb + 1) * 128],
                rhs=qh[:, n0 : n0 + 512],
                start=True,
                stop=True,
            )
            # explicit data deps on the q/k window stores: the region tracker
            # misses some strided producer->consumer edges (HW race). The PE
            # queue is FIFO, so one dep on the first MM orders the pair.
            add_dep_helper(_raw(m), _raw(qk_store[(id(q_sb), nh * 2 + s2)]), True,
                           "scores after q window store")
            if s2 == 0:
                add_dep_helper(_raw(m), _raw(qk_store[(id(k_sb), mb // 4)]), True,
                               "scores after k window store")
            scg.append(m)
        nc.scalar.activation(pt, sc, EXP, scale=0.125)
        sc_groups.append(scg)
        xag = []
        for j in range(2):
            m = nc.tensor.matmul(
                px[:, j * 512 : (j + 1) * 512],
                lhsT=vt[:, mb, h, :],
                rhs=pt[:, j * 512 : (j + 1) * 512],
                start=(mb == 0),
                stop=(mb == MB - 1),
                skip_group_check=True,
            )
            if j == 0:
                add_dep_helper(_raw(m), _raw(vt_store[mb]), True,
                               "x-accum after vt block store")
            xag.append(m)
        xa_groups.append(xag)

    def normalize(h, nh, px, last=False):
        # evacuate px: x rows and the sums row (cross-partition PSUM->SBUF
        # copy), broadcast the sums row to 64 partitions on GpSimd, fast-
        # approx reciprocal, one fused multiply into the bf16 x tile.
        nbase = nh * 1024
        xe = work.tile([65, 1024], f32, tag="xe", name=f"xe{h}_{nh}")
        nc.vector.tensor_copy(xe, px)
        s_row = work.tile([1, 1024], f32, tag="s_row", name=f"s_row{h}_{nh}")
        nc.vector.tensor_copy(s_row, px[64:65, :])
        s_bc = work.tile([64, 1024], f32, tag="s_bc", name=f"s_bc{h}_{nh}")
        nc.gpsimd.partition_broadcast(s_bc, s_row)
        r_bc = work.tile([64, 1024], f32, tag="r_bc", name=f"r_bc{h}_{nh}")
        nc.vector.reciprocal_approx_fast(r_bc, s_bc)
        nc.vector.tensor_mul(x_sb[h][:, nbase : nbase + 1024], xe[0:64, :], r_bc)
        if "dbg_s0" in io:
            bi = h * 2 + nh
            nc.sync.dma_start(io[f"dbg_s{bi}"], s_row)
            nc.sync.dma_start(io[f"dbg_r{bi}"], r_bc)
            nc.sync.dma_start(io[f"dbg_xe{bi}"], xe)

    # out-projection with SBUF accumulation: each (nh, oc, h) chunk is a
    # transient 2-matmul PSUM group copied/added into an SBUF tile, so chunks
    # interleave freely with attention iterations.
    po_sb = {}

    def po_part(nh, oc, h):
        po = psA.tile([128, 1024], f32, tag="ps", name=f"po{nh}_{oc}_{h}")
        nbase = nh * 1024
        for j in range(2):
            nc.tensor.matmul(
                po[:, j * 512 : (j + 1) * 512],
                lhsT=wmt[h][:, oc * 128 : (oc + 1) * 128],
                rhs=x_sb[h][:, nbase + j * 512 : nbase + (j + 1) * 512],
                start=True,
                stop=True,
            )
        if h == 0:
            ot = outp.tile([128, 1024], f32, tag="ot", name=f"ot{nh}_{oc}")
            nc.vector.tensor_copy(ot, po)
            po_sb[(nh, oc)] = ot
        else:
            fin = outf.tile([128, 1024], f32, tag="fin", name=f"fin{nh}_{oc}")
            nc.vector.tensor_add(fin, po_sb.pop((nh, oc)), po)
            nc.sync.dma_start(
                io["out"][oc * 128 : (oc + 1) * 128, nh * 1024 : (nh + 1) * 1024],
                fin,
            )

    # interleaved emission: remaining projections / V^T inside block 0 (vt[k]
    # must be emitted before iteration k+1 -- xa[k] of EVERY block reads it),
    # h0 out-projection chunks in block 2/3, nh0-h1 chunks late in block 3.
    fill = {
        (0, 4): lambda: (proj_step(xk_sb, wkt_v, k_sb, 1), vt_step(4)),
        (0, 5): lambda: (vt_step(5), vt_step(6)),
        (0, 6): lambda: (vt_step(7), proj_step(xq_sb, wqt_v, q_sb, 2, bq_sb)),
        (0, 7): lambda: (vt_step(8), proj_step(xk_sb, wkt_v, k_sb, 2)),
        (0, 8): lambda: (vt_step(9), vt_step(10)),
        (0, 9): lambda: (vt_step(11), proj_step(xq_sb, wqt_v, q_sb, 3, bq_sb)),
        (0, 10): lambda: (vt_step(12), proj_step(xk_sb, wkt_v, k_sb, 3)),
        (0, 11): lambda: (vt_step(13), vt_step(14)),
        (0, 12): lambda: (vt_step(15),),
        (2, 0): lambda: po_part(0, 0, 0),
        (2, 2): lambda: po_part(0, 1, 0),
        (3, 0): lambda: po_part(1, 0, 0),
        (3, 2): lambda: po_part(1, 1, 0),
        (3, 8): lambda: po_part(0, 0, 1),
        (3, 11): lambda: po_part(0, 1, 1),
    }

    blocks = [(0, 0), (0, 1), (1, 0), (1, 1)]
    for bi, (h, nh) in enumerate(blocks):
        px = psX.tile([65, 1024], f32, tag="px", name=f"px{h}_{nh}")
        for mb in range(MB):
            if (bi, mb) in fill:
                fill[(bi, mb)]()
            attn_iter(h, nh, mb, px)
        normalize(h, nh, px)

    if "dbg_s0" in io:
        nc.sync.dma_start(io["dbg_q"], q_sb)
        nc.sync.dma_start(io["dbg_k"], k_sb)
        nc.sync.dma_start(io["dbg_vt"], vt)
        nc.sync.dma_start(io["dbg_x0"], x_sb[0])
        nc.sync.dma_start(io["dbg_x1"], x_sb[1])

    # tail: only the nh1 h1 chunks remain
    po_part(1, 0, 1)
    po_part(1, 1, 1)

    # PE run-ahead deps: [scores g+2][x-accums g] alternation so the PE
    # never waits on an exp (and the HAM clock gate stays open).
    G = len(sc_groups)
    for g in range(G):
        if g + 2 < G:
            for m in xa_groups[g]:
                add_dep_helper(_raw(m), _raw(sc_groups[g + 2][-1]), False,
                               "x-accums after scores g+2")
        if g + 3 < G:
            for m in sc_groups[g + 3]:
                add_dep_helper(_raw(m), _raw(xa_groups[g][-1]), False,
                               "scores g+3 after x-accums g")


def _build_nc():
    key = "nc"
    if key in _CACHE:
        return _CACHE[key]
    from contextlib import ExitStack

    import concourse.mybir as mybir
    import concourse.tile as tile
    from concourse import bacc

    f32 = mybir.dt.float32
    bf16 = mybir.dt.bfloat16
    nc = bacc.Bacc("TRN2", target_bir_lowering=False, debug=False, num_devices=8)
    io = {}
    for name, shape, dt_ in (
        ("xq", [256, 2048], bf16),
        ("xk", [256, 2048], bf16),
        ("xv", [256, 2048], bf16),
        ("wblob", [128, 1280], bf16),
        ("bq", [128, 1], f32),
    ):
        io[name] = nc.dram_tensor(name, shape, dt_, kind="ExternalInput").ap()
    io["out"] = nc.dram_tensor("out", [256, 2048], f32, kind="ExternalOutput").ap()
    import os
    if os.environ.get("KDBG", "0") == "1":
        bf16_, f32_ = bf16, f32
        for bi in range(4):
            io[f"dbg_s{bi}"] = nc.dram_tensor(f"dbg_s{bi}", [1, 1024], f32_, kind="ExternalOutput").ap()
            io[f"dbg_r{bi}"] = nc.dram_tensor(f"dbg_r{bi}", [64, 1024], f32_, kind="ExternalOutput").ap()
            io[f"dbg_xe{bi}"] = nc.dram_tensor(f"dbg_xe{bi}", [64, 1024], f32_, kind="ExternalOutput").ap()
        io["dbg_q"] = nc.dram_tensor("dbg_q", [128, 2048], bf16_, kind="ExternalOutput").ap()
        io["dbg_k"] = nc.dram_tensor("dbg_k", [128, 2048], bf16_, kind="ExternalOutput").ap()
        io["dbg_vt"] = nc.dram_tensor("dbg_vt", [128, 16, 2, 65], bf16_, kind="ExternalOutput").ap()
        io["dbg_x0"] = nc.dram_tensor("dbg_x0", [64, 2048], bf16_, kind="ExternalOutput").ap()
        io["dbg_x1"] = nc.dram_tensor("dbg_x1", [64, 2048], bf16_, kind="ExternalOutput").ap()

    with tile.TileContext(nc) as tc:
        with ExitStack() as ctx:
            _emit(ctx, tc, io)
    nc.compile()
    _CACHE[key] = nc
    return nc


def make_in_maps(query, key, value, wq, bq, wk, bk, wv, bv, wm, bm):
    fb = lambda a: np.ascontiguousarray(np.asarray(a, dtype=np.float32)).astype(BF)
    f = lambda a: np.ascontiguousarray(np.asarray(a), dtype=np.float32)
    query, key, value = f(query), f(key), f(value)
    wq, wk, wv, wm = f(wq), f(wk), f(wv), f(wm)
    bq = f(bq)

    def chan_pack(w, idx):
        # [256,128] (chan, out) -> [128, 2, 128] -> [128, 256] packed
        wt = w[idx].T.reshape(2, 128, 128).transpose(1, 0, 2).reshape(128, 256)
        return wt

    in_maps = []
    for c in range(8):
        b, pair = divmod(c, 2)
        hs = (2 * pair, 2 * pair + 1)
        idx = np.array([d * H + h for h in hs for d in range(DIM)])
        blob = np.concatenate(
            [
                chan_pack(wq, idx),
                chan_pack(wk, idx),
                chan_pack(wv, idx),
                np.pad(
                    np.concatenate([wm[:, idx[:64]].T, wm[:, idx[64:]].T], axis=1),
                    ((0, 64), (0, 0)),
                ),
            ],
            axis=1,
        )
        m = {
            "xq": fb(query[b]),
            "xk": fb(key[b]),
            "xv": fb(value[b]),
            "wblob": fb(blob),
            "bq": f(bq[idx].reshape(128, 1)),
        }
        in_maps.append(m)
    return in_maps


def run(in_maps, trace=False, **kw):
    from concourse import bass_utils

    nc = _build_nc()
    return bass_utils.run_bass_kernel_spmd(
        nc, in_maps, core_ids=list(range(8)), trace=trace, **kw
    )


def gather(results, wm, bv, bm):
    wm = np.asarray(wm, dtype=np.float32)
    bv = np.asarray(bv, dtype=np.float32)
    bm = np.asarray(bm, dtype=np.float32)
    bias = bm + wm @ bv  # device skips bv; its output contribution is wm @ bv
    outs = [np.asarray(r["out"], dtype=np.float32) for r in results]
    return np.stack([outs[2 * b] + outs[2 * b + 1] + bias[:, None] for b in range(B)])


def kernel(query, key, value, wq, bq, wk, bk, wv, bv, wm, bm):
    in_maps = make_in_maps(query, key, value, wq, bq, wk, bk, wv, bv, wm, bm)
    res = run(in_maps)
    return gather(res.results, wm, bv, bm)


# revision 28
# speedup vs baseline: 1.0436x; 1.0085x over previous
"""MultiHeadedAttention Trainium2 Bass kernel.

Full inputs in, full output out. Sharding: 8 cores = 4 batches x 2 head-pairs
(data-parallel over batch, tensor-parallel over the 4 heads). Per core, all
matmuls in bf16 (fp32 PSUM accumulation):
  Q/K projections for its 2 heads      -> [128, 2048] bf16 (chan-major)
  V projection directly transposed     -> vt [m, (h, d+ones)] bf16
  per head/n-half: scoresT[m,n] = K^T Q, exp on ACT (scale=1/8), x[d+1, n]
  accumulated over m in PSUM with vt stationary (ones row gives softmax
  sums), normalize via DMA-broadcast + fast-approx reciprocal, out
  projection per n-half with both heads accumulated in PSUM.

Bias algebra (lets the device skip bk/bv entirely):
  - K bias shifts scores by a per-query constant -> softmax-invariant, drop.
  - V bias adds exactly bv to the normalized x (softmax weights sum to 1),
    so its output contribution is wm @ bv -> host adds (bm + wm @ bv).

Schedule notes:
  - The softmax exp on the Scalar engine (64 x [128,1024] ACTIVATEs ~71us)
    is the roofline; everything else must hide behind it.
  - PE runs 2 score-tiles AHEAD of the exp stream (deps force the static
    order [scores g+2][x-accums g]) so the PE never waits on an exp; a PE
    that waits each iteration can never assemble the ~3.4us of sustained
    activity the HAM clock gate needs to release 2.4 GHz.
  - All weights travel in ONE packed [128,1024] DMA; inputs are 3 big tiles
    loaded in 2 half-tensor DMAs each (DMA *issue* costs ~0.7us of engine
    queue time per dma_start, so fewer/bigger is better for the prologue).
  - px (unnormalized x + sums row) is evacuated to SBUF in one [65,1024]
    copy at block end so the single px PSUM buffer frees quickly.
  - Remaining projections / V^T blocks / the first out-projection group are
    emitted inside the attention loop so the static PE order interleaves
    them into PE slack instead of stalling the queue on input DMAs.
"""

import sys

if "/opt/trn_rl_repo" not in sys.path:
    sys.path.insert(0, "/opt/trn_rl_repo")

import numpy as np
import ml_dtypes

BF = ml_dtypes.bfloat16

B, D, N, H = 4, 256, 2048, 4
DIM = D // H  # 64
NW = 4  # 512-wide n windows
MB = 16  # 128-wide m blocks

_CACHE = {}


def _emit(ctx, tc, io):
    import concourse.bass as bass
    import concourse.mybir as mybir

    nc = tc.nc
    f32 = mybir.dt.float32
    bf16 = mybir.dt.bfloat16
    EXP = mybir.ActivationFunctionType.Exp

    from concourse.tile_rust import add_dep_helper

    def _raw(inst):
        return getattr(inst, "ins", inst)

    const = ctx.enter_context(tc.tile_pool(name="const", bufs=1))
    xin = ctx.enter_context(tc.tile_pool(name="xin", bufs=1))
    big = ctx.enter_context(tc.tile_pool(name="big", bufs=1))
    xpool = ctx.enter_context(tc.tile_pool(name="xpool", bufs=2))
    pb = ctx.enter_context(tc.tile_pool(name="probs", bufs=4))
    work = ctx.enter_context(tc.tile_pool(name="work", bufs=2))
    outp = ctx.enter_context(tc.tile_pool(name="outp", bufs=4))
    outf = ctx.enter_context(tc.tile_pool(name="outf", bufs=2))
    psA = ctx.enter_context(tc.tile_pool(name="psA", bufs=3, space="PSUM"))
    psX = ctx.enter_context(tc.tile_pool(name="psX", bufs=1, space="PSUM"))
    dpool = ctx.enter_context(tc.tile_pool(name="dpool", bufs=2, space="DRAM"))

    # ---- input loads: 3 big tiles, 2 half-tensor DMAs each, n-half A first
    xq_sb = xin.tile([128, 2, 2048], bf16, tag="xq")
    xk_sb = xin.tile([128, 2, 2048], bf16, tag="xk")
    xv_sb = xin.tile([128, 2, 2048], bf16, tag="xv")

    def load_cols(eng, t, name, c0, c1):
        src = io[name].rearrange("(c p) n -> p c n", p=128)
        s = slice(c0, c1)
        eng.dma_start(t[:, :, s], src[:, :, s])

    # weights first on the scalar ring (small), leading input chunks fine-
    # grained on the sync ring so the first projections start ASAP; the two
    # HWDGE rings run in parallel.
    wblob = const.tile([128, 1280], bf16, tag="wblob")
    nc.scalar.dma_start(wblob, io["wblob"])
    bq_sb = const.tile([128, 1], f32, tag="bq")
    nc.scalar.dma_start(bq_sb, io["bq"])

    load_cols(nc.sync, xk_sb, "xk", 0, 512)
    load_cols(nc.sync, xq_sb, "xq", 0, 512)
    load_cols(nc.scalar, xv_sb, "xv", 0, 512)
    load_cols(nc.scalar, xq_sb, "xq", 512, 1024)
    load_cols(nc.sync, xk_sb, "xk", 512, 1024)
    load_cols(nc.scalar, xv_sb, "xv", 512, 1024)
    load_cols(nc.sync, xq_sb, "xq", 1024, 2048)
    load_cols(nc.scalar, xk_sb, "xk", 1024, 2048)
    load_cols(nc.sync, xv_sb, "xv", 1024, 2048)
    wqt_v = wblob[:, 0:256].rearrange("p (c o) -> p c o", c=2)
    wkt_v = wblob[:, 256:512].rearrange("p (c o) -> p c o", c=2)
    wvt_v = wblob[:, 512:768].rearrange("p (c o) -> p c o", c=2)
    wmt = (wblob[0:64, 768:1024], wblob[0:64, 1024:1280])

    # ---- small constants ----
    wu_a = const.tile([128, 128], bf16, tag="wu_a")
    nc.gpsimd.memset(wu_a, 0.0)
    wu_b = const.tile([128, 512], bf16, tag="wu_b")
    nc.gpsimd.memset(wu_b, 0.0)
    dum = const.tile([1, 16], f32, tag="dum")
    nc.gpsimd.memset(dum, 0.0)
    ones64 = const.tile([1, 64], f32, tag="ones64")
    nc.gpsimd.memset(ones64, 1.0)
    dum_o = const.tile([1, 16], bf16, tag="dum_o")
    # dummy exp: forces the ~2.7us ACT table load during the DMA ramp
    nc.scalar.activation(dum_o, dum, EXP, scale=0.125)

    # PE warmup: HAM clock gate releases (1.2 -> 2.4 GHz) after ~3.4us of
    # sustained matmul activity; the cold 512-wide matmuls bridge the input
    # DMA ramp so the projections start near-warm.
    wu_ps = psA.tile([128, 1024], f32, tag="ps", name="wu_ps")
    for i in range(8):
        nc.tensor.matmul(wu_ps[:, 0:512], lhsT=wu_a, rhs=wu_b, start=True, stop=True)

    # ---- projections (Q with bias, K bias dropped) + V^T ----
    q_sb = big.tile([128, 2048], bf16, tag="q")
    k_sb = big.tile([128, 2048], bf16, tag="k")

    qk_store = {}  # (dst_tag, window) -> producing DVE instruction
    vt_store = {}  # mb -> producing DVE instruction

    def proj_step(xt, wt, dst, w, bias=None):
        ps = psA.tile([128, 1024], f32, tag="ps", name=f"psproj{w}")
        s = slice(w * 512, (w + 1) * 512)
        nc.tensor.matmul(ps[:, 0:512], lhsT=wt[:, 0, :], rhs=xt[:, 0, s], start=True, stop=False)
        nc.tensor.matmul(ps[:, 0:512], lhsT=wt[:, 1, :], rhs=xt[:, 1, s], start=False, stop=True)
        d = dst[:, s]
        if bias is None:
            st = nc.vector.tensor_copy(d, ps[:, 0:512])
        else:
            st = nc.vector.tensor_scalar_add(d, ps[:, 0:512], bias)
        qk_store[(id(dst), w)] = st

    vt = big.tile([128, MB, 2, 65], bf16, tag="vt")
    nc.gpsimd.memset(vt[:, :, :, 64:65], 1.0)

    def vt_step(mb):
        ms = slice(mb * 128, (mb + 1) * 128)
        ps = psA.tile([128, 1024], f32, tag="ps", name=f"psvt{mb}")
        pvt = ps[:, 0:128]
        nc.tensor.matmul(pvt, lhsT=xv_sb[:, 0, ms], rhs=wvt_v[:, 0, :], start=True, stop=False)
        nc.tensor.matmul(pvt, lhsT=xv_sb[:, 1, ms], rhs=wvt_v[:, 1, :], start=False, stop=True)
        vt_store[mb] = nc.vector.tensor_copy(
            vt[:, mb, :, 0:64], pvt.rearrange("m (h d) -> m h d", h=2)
        )

    proj_step(xk_sb, wkt_v, k_sb, 0)
    proj_step(xq_sb, wqt_v, q_sb, 0, bq_sb)
    proj_step(xq_sb, wqt_v, q_sb, 1, bq_sb)
    for mb in range(4):
        vt_step(mb)

    # ---- attention ----
    x_sb = [xpool.tile([64, 2048], bf16, tag="x", name=f"x{h}") for h in range(2)]
    sc_groups = []
    xa_groups = []
    grp_meta = []
    po_state = {}

    def attn_iter(h, nh, mb, px):
        qh = q_sb[h * 64 : (h + 1) * 64, :]
        kh = k_sb[h * 64 : (h + 1) * 64, :]
        nbase = nh * 1024
        pt = pb.tile([128, 1024], bf16, tag="pt", name="pt")
        sc = psA.tile([128, 1024], f32, tag="ps", name="pssc")
        scg = []
        for s2 in range(2):
            n0 = nbase + s2 * 512
            m = nc.tensor.matmul(
                sc[:, s2 * 512 : (s2 + 1) * 512],
                lhsT=kh[:, mb * 128 : (mb + 1) * 128],
                rhs=qh[:, n0 : n0 + 512],
                start=True,
                stop=True,
            )
            scg.append(m)
        nc.scalar.activation(pt, sc, EXP, scale=0.125)
        sc_groups.append(scg)
        xag = []
        for j in range(2):
            m = nc.tensor.matmul(
                px[:, j * 512 : (j + 1) * 512],
                lhsT=vt[:, mb, h, :],
                rhs=pt[:, j * 512 : (j + 1) * 512],
                start=(mb == 0),
                stop=(mb == MB - 1),
                skip_group_check=True,
            )
            xag.append(m)
        xa_groups.append(xag)
        grp_meta.append((nh, mb))

    def normalize(h, nh, px, last=False):
        # evacuate px: x rows and the sums row (cross-partition PSUM->SBUF
        # copy), broadcast the sums row to 64 partitions on GpSimd, fast-
        # approx reciprocal, one fused multiply into the bf16 x tile.
        nbase = nh * 1024
        xe = work.tile([65, 1024], f32, tag="xe", name=f"xe{h}_{nh}")
        nc.vector.tensor_copy(xe, px)
        # sums row copied on ACT, in parallel with the DVE xe copy, so px
        # frees after ~1.1us instead of two serial DVE copies (the block-
        # boundary PE bubble is what re-throttles the HAM clock gate).
        s_row = work.tile([1, 1024], f32, tag="s_row", name=f"s_row{h}_{nh}")
        nc.scalar.copy(s_row, px[64:65, :])
        s_bc = work.tile([64, 1024], f32, tag="s_bc", name=f"s_bc{h}_{nh}")
        nc.gpsimd.partition_broadcast(s_bc, s_row)
        r_bc = work.tile([64, 1024], f32, tag="r_bc", name=f"r_bc{h}_{nh}")
        nc.vector.reciprocal_approx_fast(r_bc, s_bc)
        nc.vector.tensor_mul(
            x_sb[h][:, nbase : nbase + 1024], xe[0:64, :], r_bc
        )
        if "dbg_s0" in io:
            bi = h * 2 + nh
            nc.sync.dma_start(io[f"dbg_s{bi}"], s_row)
            nc.sync.dma_start(io[f"dbg_r{bi}"], r_bc)
            nc.sync.dma_start(io[f"dbg_xe{bi}"], xe)

    # out-projection with SBUF accumulation: each (nh, oc, h) chunk is a
    # transient 2-matmul PSUM group copied/added into an SBUF tile, so chunks
    # interleave freely with attention iterations.
    po_sb = {}

    def po_part(nh, oc, h):
        po = psA.tile([128, 1024], f32, tag="ps", name=f"po{nh}_{oc}_{h}")
        nbase = nh * 1024
        for j in range(2):
            nc.tensor.matmul(
                po[:, j * 512 : (j + 1) * 512],
                lhsT=wmt[h][:, oc * 128 : (oc + 1) * 128],
                rhs=x_sb[h][:, nbase + j * 512 : nbase + (j + 1) * 512],
                start=True,
                stop=True,
            )
        if h == 0:
            ot = outp.tile([128, 1024], f32, tag="ot", name=f"ot{nh}_{oc}")
            nc.vector.tensor_copy(ot, po)
            po_sb[(nh, oc)] = ot
        else:
            fin = outf.tile([128, 1024], f32, tag="fin", name=f"fin{nh}_{oc}")
            nc.vector.tensor_add(fin, po_sb.pop((nh, oc)), po)
            nc.sync.dma_start(
                io["out"][oc * 128 : (oc + 1) * 128, nh * 1024 : (nh + 1) * 1024],
                fin,
            )

    # interleaved emission: remaining projections / V^T inside block 0 (vt[k]
    # must be emitted before iteration k+1 -- xa[k] of EVERY block reads it),
    # h0 out-projection chunks in block 2/3, nh0-h1 chunks late in block 3.
    fill = {
        (0, 4): lambda: (proj_step(xk_sb, wkt_v, k_sb, 1), vt_step(4)),
        (0, 5): lambda: (vt_step(5), vt_step(6)),
        (0, 6): lambda: (vt_step(7), proj_step(xq_sb, wqt_v, q_sb, 2, bq_sb)),
        (0, 7): lambda: (vt_step(8), proj_step(xk_sb, wkt_v, k_sb, 2)),
        (0, 8): lambda: (vt_step(9), vt_step(10)),
        (0, 9): lambda: (vt_step(11), proj_step(xq_sb, wqt_v, q_sb, 3, bq_sb)),
        (0, 10): lambda: (vt_step(12), proj_step(xk_sb, wkt_v, k_sb, 3)),
        (0, 11): lambda: (vt_step(13), vt_step(14)),
        (0, 12): lambda: (vt_step(15),),
        (2, 0): lambda: po_part(0, 0, 0),
        (2, 2): lambda: po_part(0, 1, 0),
        (3, 0): lambda: po_part(1, 0, 0),
        (3, 2): lambda: po_part(1, 1, 0),
        (3, 8): lambda: po_part(0, 0, 1),
        (3, 11): lambda: po_part(0, 1, 1),
    }

    blocks = [(0, 0), (0, 1), (1, 0), (1, 1)]
    for bi, (h, nh) in enumerate(blocks):
        px = psX.tile([65, 1024], f32, tag="px", name=f"px{h}_{nh}")
        for mb in range(MB):
            if (bi, mb) in fill:
                fill[(bi, mb)]()
            attn_iter(h, nh, mb, px)
        normalize(h, nh, px)

    if "dbg_s0" in io:
        nc.sync.dma_start(io["dbg_q"], q_sb)
        nc.sync.dma_start(io["dbg_k"], k_sb)
        nc.sync.dma_start(io["dbg_vt"], vt)
        nc.sync.dma_start(io["dbg_x0"], x_sb[0])
        nc.sync.dma_start(io["dbg_x1"], x_sb[1])

    # tail: only the nh1 h1 chunks remain
    po_part(1, 0, 1)
    po_part(1, 1, 1)

    # explicit data deps on the q/k window and vt block stores: the region
    # tracker misses some strided producer->consumer edges (HW race). The PE
    # queue is FIFO, so a dep on the first MM of a pair orders both.
    for g, (nh_g, mb_g) in enumerate(grp_meta):
        add_dep_helper(_raw(sc_groups[g][0]), _raw(qk_store[(id(k_sb), mb_g // 4)]),
                       True, "scores after k window store")
        for s2 in range(2):
            add_dep_helper(_raw(sc_groups[g][s2]),
                           _raw(qk_store[(id(q_sb), nh_g * 2 + s2)]),
                           True, "scores after q window store")
        add_dep_helper(_raw(xa_groups[g][0]), _raw(vt_store[mb_g]), True,
                       "x-accum after vt block store")

    # PE run-ahead deps: [scores g+2][x-accums g] alternation so the PE
    # never waits on an exp (and the HAM clock gate stays open).
    G = len(sc_groups)
    for g in range(G):
        if g + 2 < G:
            for m in xa_groups[g]:
                add_dep_helper(_raw(m), _raw(sc_groups[g + 2][-1]), False,
                               "x-accums after scores g+2")
        if g + 3 < G:
            for m in sc_groups[g + 3]:
                add_dep_helper(_raw(m), _raw(xa_groups[g][-1]), False,
                               "scores g+3 after x-accums g")


def _build_nc():
    key = "nc"
    if key in _CACHE:
        return _CACHE[key]
    from contextlib import ExitStack

    import concourse.mybir as mybir
    import concourse.tile as tile
    from concourse import bacc

    f32 = mybir.dt.float32
    bf16 = mybir.dt.bfloat16
    nc = bacc.Bacc("TRN2", target_bir_lowering=False, debug=False, num_devices=8)
    io = {}
    for name, shape, dt_ in (
        ("xq", [256, 2048], bf16),
        ("xk", [256, 2048], bf16),
        ("xv", [256, 2048], bf16),
        ("wblob", [128, 1280], bf16),
        ("bq", [128, 1], f32),
    ):
        io[name] = nc.dram_tensor(name, shape, dt_, kind="ExternalInput").ap()
    io["out"] = nc.dram_tensor("out", [256, 2048], f32, kind="ExternalOutput").ap()
    import os
    if os.environ.get("KDBG", "0") == "1":
        bf16_, f32_ = bf16, f32
        for bi in range(4):
            io[f"dbg_s{bi}"] = nc.dram_tensor(f"dbg_s{bi}", [1, 1024], f32_, kind="ExternalOutput").ap()
            io[f"dbg_r{bi}"] = nc.dram_tensor(f"dbg_r{bi}", [64, 1024], f32_, kind="ExternalOutput").ap()
            io[f"dbg_xe{bi}"] = nc.dram_tensor(f"dbg_xe{bi}", [64, 1024], f32_, kind="ExternalOutput").ap()
        io["dbg_q"] = nc.dram_tensor("dbg_q", [128, 2048], bf16_, kind="ExternalOutput").ap()
        io["dbg_k"] = nc.dram_tensor("dbg_k", [128, 2048], bf16_, kind="ExternalOutput").ap()
        io["dbg_vt"] = nc.dram_tensor("dbg_vt", [128, 16, 2, 65], bf16_, kind="ExternalOutput").ap()
        io["dbg_x0"] = nc.dram_tensor("dbg_x0", [64, 2048], bf16_, kind="ExternalOutput").ap()
        io["dbg_x1"] = nc.dram_tensor("dbg_x1", [64, 2048], bf16_, kind="ExternalOutput").ap()

    with tile.TileContext(nc) as tc:
        with ExitStack() as ctx:
            _emit(ctx, tc, io)
    nc.compile()
    _CACHE[key] = nc
    return nc


def make_in_maps(query, key, value, wq, bq, wk, bk, wv, bv, wm, bm):
    fb = lambda a: np.ascontiguousarray(np.asarray(a, dtype=np.float32)).astype(BF)
    f = lambda a: np.ascontiguousarray(np.asarray(a), dtype=np.float32)
    query, key, value = f(query), f(key), f(value)
    wq, wk, wv, wm = f(wq), f(wk), f(wv), f(wm)
    bq = f(bq)

    def chan_pack(w, idx):
        # [256,128] (chan, out) -> [128, 2, 128] -> [128, 256] packed
        wt = w[idx].T.reshape(2, 128, 128).transpose(1, 0, 2).reshape(128, 256)
        return wt

    in_maps = []
    for c in range(8):
        b, pair = divmod(c, 2)
        hs = (2 * pair, 2 * pair + 1)
        idx = np.array([d * H + h for h in hs for d in range(DIM)])
        blob = np.concatenate(
            [
                chan_pack(wq, idx),
                chan_pack(wk, idx),
                chan_pack(wv, idx),
                np.pad(
                    np.concatenate([wm[:, idx[:64]].T, wm[:, idx[64:]].T], axis=1),
                    ((0, 64), (0, 0)),
                ),
            ],
            axis=1,
        )
        m = {
            "xq": fb(query[b]),
            "xk": fb(key[b]),
            "xv": fb(value[b]),
            "wblob": fb(blob),
            "bq": f(bq[idx].reshape(128, 1)),
        }
        in_maps.append(m)
    return in_maps


def run(in_maps, trace=False, **kw):
    from concourse import bass_utils

    nc = _build_nc()
    return bass_utils.run_bass_kernel_spmd(
        nc, in_maps, core_ids=list(range(8)), trace=trace, **kw
    )


def gather(results, wm, bv, bm):
    wm = np.asarray(wm, dtype=np.float32)
    bv = np.asarray(bv, dtype=np.float32)
    bm = np.asarray(bm, dtype=np.float32)
    bias = bm + wm @ bv  # device skips bv; its output contribution is wm @ bv
    outs = [np.asarray(r["out"], dtype=np.float32) for r in results]
    return np.stack([outs[2 * b] + outs[2 * b + 1] + bias[:, None] for b in range(B)])


def kernel(query, key, value, wq, bq, wk, bk, wv, bv, wm, bm):
    in_maps = make_in_maps(query, key, value, wq, bq, wk, bk, wv, bv, wm, bm)
    res = run(in_maps)
    return gather(res.results, wm, bv, bm)


# revision 29
# speedup vs baseline: 1.1036x; 1.0575x over previous
"""MultiHeadedAttention Trainium2 Bass kernel.

Full inputs in, full output out. Sharding: 8 cores = 4 batches x 2 head-pairs
(data-parallel over batch, tensor-parallel over the 4 heads). Per core, all
matmuls in bf16 (fp32 PSUM accumulation):
  Q/K projections for its 2 heads      -> [128, 2048] bf16 (chan-major)
  V projection directly transposed     -> vt [m, (h, d+ones)] bf16
  per head/n-half: scoresT[m,n] = K^T Q, exp on ACT (scale=1/8), x[d+1, n]
  accumulated over m in PSUM with vt stationary (ones row gives softmax
  sums), normalize via DMA-broadcast + fast-approx reciprocal, out
  projection per n-half with both heads accumulated in PSUM.

Bias algebra (lets the device skip bk/bv entirely):
  - K bias shifts scores by a per-query constant -> softmax-invariant, drop.
  - V bias adds exactly bv to the normalized x (softmax weights sum to 1),
    so its output contribution is wm @ bv -> host adds (bm + wm @ bv).

Schedule notes:
  - The softmax exp on the Scalar engine (64 x [128,1024] ACTIVATEs ~71us)
    is the roofline; everything else must hide behind it.
  - PE runs 2 score-tiles AHEAD of the exp stream (deps force the static
    order [scores g+2][x-accums g]) so the PE never waits on an exp; a PE
    that waits each iteration can never assemble the ~3.4us of sustained
    activity the HAM clock gate needs to release 2.4 GHz.
  - All weights travel in ONE packed [128,1024] DMA; inputs are 3 big tiles
    loaded in 2 half-tensor DMAs each (DMA *issue* costs ~0.7us of engine
    queue time per dma_start, so fewer/bigger is better for the prologue).
  - px (unnormalized x + sums row) is evacuated to SBUF in one [65,1024]
    copy at block end so the single px PSUM buffer frees quickly.
  - Remaining projections / V^T blocks / the first out-projection group are
    emitted inside the attention loop so the static PE order interleaves
    them into PE slack instead of stalling the queue on input DMAs.
"""

import sys

if "/opt/trn_rl_repo" not in sys.path:
    sys.path.insert(0, "/opt/trn_rl_repo")

import numpy as np
import ml_dtypes

BF = ml_dtypes.bfloat16

B, D, N, H = 4, 256, 2048, 4
DIM = D // H  # 64
NW = 4  # 512-wide n windows
MB = 16  # 128-wide m blocks

_CACHE = {}


def _emit(ctx, tc, io):
    import concourse.bass as bass
    import concourse.mybir as mybir

    nc = tc.nc
    f32 = mybir.dt.float32
    bf16 = mybir.dt.bfloat16
    EXP = mybir.ActivationFunctionType.Exp

    from concourse.tile_rust import add_dep_helper

    def _raw(inst):
        return getattr(inst, "ins", inst)

    const = ctx.enter_context(tc.tile_pool(name="const", bufs=1))
    xin = ctx.enter_context(tc.tile_pool(name="xin", bufs=1))
    big = ctx.enter_context(tc.tile_pool(name="big", bufs=1))
    xpool = ctx.enter_context(tc.tile_pool(name="xpool", bufs=2))
    pb = ctx.enter_context(tc.tile_pool(name="probs", bufs=4))
    work = ctx.enter_context(tc.tile_pool(name="work", bufs=2))
    outp = ctx.enter_context(tc.tile_pool(name="outp", bufs=4))
    outf = ctx.enter_context(tc.tile_pool(name="outf", bufs=2))
    psA = ctx.enter_context(tc.tile_pool(name="psA", bufs=3, space="PSUM"))
    psX = ctx.enter_context(tc.tile_pool(name="psX", bufs=1, space="PSUM"))
    dpool = ctx.enter_context(tc.tile_pool(name="dpool", bufs=2, space="DRAM"))

    # ---- input loads: 3 big tiles, 2 half-tensor DMAs each, n-half A first
    xq_sb = xin.tile([128, 2, 2048], bf16, tag="xq")
    xk_sb = xin.tile([128, 2, 2048], bf16, tag="xk")
    xv_sb = xin.tile([128, 2, 2048], bf16, tag="xv")

    def load_cols(eng, t, name, c0, c1):
        src = io[name].rearrange("(c p) n -> p c n", p=128)
        s = slice(c0, c1)
        eng.dma_start(t[:, :, s], src[:, :, s])

    # weights first on the scalar ring (small), leading input chunks fine-
    # grained on the sync ring so the first projections start ASAP; the two
    # HWDGE rings run in parallel.
    wblob = const.tile([128, 1280], bf16, tag="wblob")
    nc.scalar.dma_start(wblob, io["wblob"])
    bq_sb = const.tile([128, 1], f32, tag="bq")
    nc.scalar.dma_start(bq_sb, io["bq"])

    load_cols(nc.sync, xk_sb, "xk", 0, 512)
    load_cols(nc.sync, xq_sb, "xq", 0, 512)
    load_cols(nc.scalar, xv_sb, "xv", 0, 512)
    load_cols(nc.scalar, xq_sb, "xq", 512, 1024)
    load_cols(nc.sync, xk_sb, "xk", 512, 1024)
    load_cols(nc.scalar, xv_sb, "xv", 512, 1024)
    load_cols(nc.sync, xq_sb, "xq", 1024, 2048)
    load_cols(nc.scalar, xk_sb, "xk", 1024, 2048)
    load_cols(nc.sync, xv_sb, "xv", 1024, 2048)
    wqt_v = wblob[:, 0:256].rearrange("p (c o) -> p c o", c=2)
    wkt_v = wblob[:, 256:512].rearrange("p (c o) -> p c o", c=2)
    wvt_v = wblob[:, 512:768].rearrange("p (c o) -> p c o", c=2)
    wmt = (wblob[0:64, 768:1024], wblob[0:64, 1024:1280])

    # ---- small constants ----
    wu_a = const.tile([128, 128], bf16, tag="wu_a")
    nc.gpsimd.memset(wu_a, 0.0)
    wu_b = const.tile([128, 512], bf16, tag="wu_b")
    nc.gpsimd.memset(wu_b, 0.0)
    dum = const.tile([1, 16], f32, tag="dum")
    nc.gpsimd.memset(dum, 0.0)
    ones64 = const.tile([1, 64], f32, tag="ones64")
    nc.gpsimd.memset(ones64, 1.0)
    dum_o = const.tile([1, 16], bf16, tag="dum_o")
    # dummy exp: forces the ~2.7us ACT table load during the DMA ramp
    nc.scalar.activation(dum_o, dum, EXP, scale=0.125)

    # PE warmup: HAM clock gate releases (1.2 -> 2.4 GHz) after ~3.4us of
    # sustained matmul activity; the cold 512-wide matmuls bridge the input
    # DMA ramp so the projections start near-warm.
    wu_ps = psA.tile([128, 1024], f32, tag="ps", name="wu_ps")
    for i in range(8):
        nc.tensor.matmul(wu_ps[:, 0:512], lhsT=wu_a, rhs=wu_b, start=True, stop=True)

    # ---- projections (Q with bias, K bias dropped) + V^T ----
    q_sb = big.tile([128, 2048], bf16, tag="q")
    k_sb = big.tile([128, 2048], bf16, tag="k")

    qk_store = {}  # (dst_tag, window) -> producing DVE instruction
    vt_store = {}  # mb -> producing DVE instruction

    def proj_step(xt, wt, dst, w, bias=None):
        ps = psA.tile([128, 1024], f32, tag="ps", name=f"psproj{w}")
        s = slice(w * 512, (w + 1) * 512)
        nc.tensor.matmul(ps[:, 0:512], lhsT=wt[:, 0, :], rhs=xt[:, 0, s], start=True, stop=False)
        nc.tensor.matmul(ps[:, 0:512], lhsT=wt[:, 1, :], rhs=xt[:, 1, s], start=False, stop=True)
        d = dst[:, s]
        if bias is None:
            st = nc.vector.tensor_copy(d, ps[:, 0:512])
        else:
            st = nc.vector.tensor_scalar_add(d, ps[:, 0:512], bias)
        qk_store[(id(dst), w)] = st

    vt = big.tile([128, MB, 2, 65], bf16, tag="vt")
    nc.gpsimd.memset(vt[:, :, :, 64:65], 1.0)

    def vt_step(mb):
        ms = slice(mb * 128, (mb + 1) * 128)
        ps = psA.tile([128, 1024], f32, tag="ps", name=f"psvt{mb}")
        pvt = ps[:, 0:128]
        nc.tensor.matmul(pvt, lhsT=xv_sb[:, 0, ms], rhs=wvt_v[:, 0, :], start=True, stop=False)
        nc.tensor.matmul(pvt, lhsT=xv_sb[:, 1, ms], rhs=wvt_v[:, 1, :], start=False, stop=True)
        vt_store[mb] = nc.vector.tensor_copy(
            vt[:, mb, :, 0:64], pvt.rearrange("m (h d) -> m h d", h=2)
        )

    proj_step(xk_sb, wkt_v, k_sb, 0)
    proj_step(xq_sb, wqt_v, q_sb, 0, bq_sb)
    proj_step(xq_sb, wqt_v, q_sb, 1, bq_sb)
    for mb in range(4):
        vt_step(mb)

    # ---- attention ----
    x_sb = [xpool.tile([64, 2048], bf16, tag="x", name=f"x{h}") for h in range(2)]
    sc_groups = []
    xa_groups = []
    grp_meta = []
    po_state = {}

    def attn_iter(h, nh, mb, px):
        qh = q_sb[h * 64 : (h + 1) * 64, :]
        kh = k_sb[h * 64 : (h + 1) * 64, :]
        nbase = nh * 1024
        pt = pb.tile([128, 1024], bf16, tag="pt", name="pt")
        sc = psA.tile([128, 1024], f32, tag="ps", name="pssc")
        scg = []
        for s2 in range(2):
            n0 = nbase + s2 * 512
            m = nc.tensor.matmul(
                sc[:, s2 * 512 : (s2 + 1) * 512],
                lhsT=kh[:, mb * 128 : (mb + 1) * 128],
                rhs=qh[:, n0 : n0 + 512],
                start=True,
                stop=True,
            )
            scg.append(m)
        nc.scalar.activation(pt, sc, EXP, scale=0.125)
        sc_groups.append(scg)
        xag = []
        for j in range(2):
            m = nc.tensor.matmul(
                px[:, j * 512 : (j + 1) * 512],
                lhsT=vt[:, mb, h, :],
                rhs=pt[:, j * 512 : (j + 1) * 512],
                start=(mb == 0),
                stop=(mb == MB - 1),
                skip_group_check=True,
            )
            xag.append(m)
        xa_groups.append(xag)
        grp_meta.append((nh, mb))

    def normalize(h, nh, px, last=False):
        # evacuate px: x rows and the sums row (cross-partition PSUM->SBUF
        # copy), broadcast the sums row to 64 partitions on GpSimd, fast-
        # approx reciprocal, one fused multiply into the bf16 x tile.
        nbase = nh * 1024
        xe = work.tile([65, 1024], f32, tag="xe", name=f"xe{h}_{nh}")
        nc.vector.tensor_copy(xe, px)
        # sums row read from xe (not px) so px frees after ONE DVE copy --
        # the block-boundary PE bubble is what re-throttles the HAM clock.
        s_row = work.tile([1, 1024], f32, tag="s_row", name=f"s_row{h}_{nh}")
        nc.vector.tensor_copy(s_row, xe[64:65, :])
        s_bc = work.tile([64, 1024], f32, tag="s_bc", name=f"s_bc{h}_{nh}")
        nc.gpsimd.partition_broadcast(s_bc, s_row)
        r_bc = work.tile([64, 1024], f32, tag="r_bc", name=f"r_bc{h}_{nh}")
        nc.vector.reciprocal_approx_fast(r_bc, s_bc)
        nc.vector.tensor_mul(
            x_sb[h][:, nbase : nbase + 1024], xe[0:64, :], r_bc
        )
        if "dbg_s0" in io:
            bi = h * 2 + nh
            nc.sync.dma_start(io[f"dbg_s{bi}"], s_row)
            nc.sync.dma_start(io[f"dbg_r{bi}"], r_bc)
            nc.sync.dma_start(io[f"dbg_xe{bi}"], xe)

    # out-projection with SBUF accumulation: each (nh, oc, h) chunk is a
    # transient 2-matmul PSUM group copied/added into an SBUF tile, so chunks
    # interleave freely with attention iterations.
    po_sb = {}

    def po_part(nh, oc, h):
        po = psA.tile([128, 1024], f32, tag="ps", name=f"po{nh}_{oc}_{h}")
        nbase = nh * 1024
        for j in range(2):
            nc.tensor.matmul(
                po[:, j * 512 : (j + 1) * 512],
                lhsT=wmt[h][:, oc * 128 : (oc + 1) * 128],
                rhs=x_sb[h][:, nbase + j * 512 : nbase + (j + 1) * 512],
                start=True,
                stop=True,
            )
        if h == 0:
            ot = outp.tile([128, 1024], f32, tag="ot", name=f"ot{nh}_{oc}")
            nc.vector.tensor_copy(ot, po)
            po_sb[(nh, oc)] = ot
        else:
            fin = outf.tile([128, 1024], f32, tag="fin", name=f"fin{nh}_{oc}")
            nc.vector.tensor_add(fin, po_sb.pop((nh, oc)), po)
            nc.sync.dma_start(
                io["out"][oc * 128 : (oc + 1) * 128, nh * 1024 : (nh + 1) * 1024],
                fin,
            )

    # interleaved emission: remaining projections / V^T inside block 0 (vt[k]
    # must be emitted before iteration k+1 -- xa[k] of EVERY block reads it),
    # h0 out-projection chunks in block 2/3, nh0-h1 chunks late in block 3.
    fill = {
        (0, 4): lambda: (proj_step(xk_sb, wkt_v, k_sb, 1), vt_step(4)),
        (0, 5): lambda: (vt_step(5), vt_step(6)),
        (0, 6): lambda: (vt_step(7), proj_step(xq_sb, wqt_v, q_sb, 2, bq_sb)),
        (0, 7): lambda: (vt_step(8), proj_step(xk_sb, wkt_v, k_sb, 2)),
        (0, 8): lambda: (vt_step(9), vt_step(10)),
        (0, 9): lambda: (vt_step(11), proj_step(xq_sb, wqt_v, q_sb, 3, bq_sb)),
        (0, 10): lambda: (vt_step(12), proj_step(xk_sb, wkt_v, k_sb, 3)),
        (0, 11): lambda: (vt_step(13), vt_step(14)),
        (0, 12): lambda: (vt_step(15),),
        (1, 5): lambda: po_part(0, 0, 0),
        (1, 9): lambda: po_part(0, 1, 0),
        (2, 5): lambda: po_part(1, 0, 0),
        (2, 9): lambda: po_part(1, 1, 0),
        (3, 5): lambda: po_part(0, 0, 1),
        (3, 9): lambda: po_part(0, 1, 1),
    }

    blocks = [(0, 0), (0, 1), (1, 0), (1, 1)]
    for bi, (h, nh) in enumerate(blocks):
        px = psX.tile([65, 1024], f32, tag="px", name=f"px{h}_{nh}")
        for mb in range(MB):
            if (bi, mb) in fill:
                fill[(bi, mb)]()
            attn_iter(h, nh, mb, px)
        normalize(h, nh, px)

    if "dbg_s0" in io:
        nc.sync.dma_start(io["dbg_q"], q_sb)
        nc.sync.dma_start(io["dbg_k"], k_sb)
        nc.sync.dma_start(io["dbg_vt"], vt)
        nc.sync.dma_start(io["dbg_x0"], x_sb[0])
        nc.sync.dma_start(io["dbg_x1"], x_sb[1])

    # tail: only the nh1 h1 chunks remain
    po_part(1, 0, 1)
    po_part(1, 1, 1)

    # explicit data deps on the q/k window and vt block stores: the region
    # tracker misses some strided producer->consumer edges (HW race). The PE
    # queue is FIFO, so a dep on the first MM of a pair orders both.
    for g, (nh_g, mb_g) in enumerate(grp_meta):
        add_dep_helper(_raw(sc_groups[g][0]), _raw(qk_store[(id(k_sb), mb_g // 4)]),
                       True, "scores after k window store")
        for s2 in range(2):
            add_dep_helper(_raw(sc_groups[g][s2]),
                           _raw(qk_store[(id(q_sb), nh_g * 2 + s2)]),
                           True, "scores after q window store")
        add_dep_helper(_raw(xa_groups[g][0]), _raw(vt_store[mb_g]), True,
                       "x-accum after vt block store")

    # PE run-ahead deps: [scores g+2][x-accums g] alternation so the PE
    # never waits on an exp (and the HAM clock gate stays open).
    G = len(sc_groups)
    for g in range(G):
        if g + 2 < G:
            for m in xa_groups[g]:
                add_dep_helper(_raw(m), _raw(sc_groups[g + 2][-1]), False,
                               "x-accums after scores g+2")
        if g + 3 < G:
            for m in sc_groups[g + 3]:
                add_dep_helper(_raw(m), _raw(xa_groups[g][-1]), False,
                               "scores g+3 after x-accums g")


def _build_nc():
    key = "nc"
    if key in _CACHE:
        return _CACHE[key]
    from contextlib import ExitStack

    import concourse.mybir as mybir
    import concourse.tile as tile
    from concourse import bacc

    f32 = mybir.dt.float32
    bf16 = mybir.dt.bfloat16
    nc = bacc.Bacc("TRN2", target_bir_lowering=False, debug=False, num_devices=8)
    io = {}
    for name, shape, dt_ in (
        ("xq", [256, 2048], bf16),
        ("xk", [256, 2048], bf16),
        ("xv", [256, 2048], bf16),
        ("wblob", [128, 1280], bf16),
        ("bq", [128, 1], f32),
    ):
        io[name] = nc.dram_tensor(name, shape, dt_, kind="ExternalInput").ap()
    io["out"] = nc.dram_tensor("out", [256, 2048], f32, kind="ExternalOutput").ap()
    import os
    if os.environ.get("KDBG", "0") == "1":
        bf16_, f32_ = bf16, f32
        for bi in range(4):
            io[f"dbg_s{bi}"] = nc.dram_tensor(f"dbg_s{bi}", [1, 1024], f32_, kind="ExternalOutput").ap()
            io[f"dbg_r{bi}"] = nc.dram_tensor(f"dbg_r{bi}", [64, 1024], f32_, kind="ExternalOutput").ap()
            io[f"dbg_xe{bi}"] = nc.dram_tensor(f"dbg_xe{bi}", [64, 1024], f32_, kind="ExternalOutput").ap()
        io["dbg_q"] = nc.dram_tensor("dbg_q", [128, 2048], bf16_, kind="ExternalOutput").ap()
        io["dbg_k"] = nc.dram_tensor("dbg_k", [128, 2048], bf16_, kind="ExternalOutput").ap()
        io["dbg_vt"] = nc.dram_tensor("dbg_vt", [128, 16, 2, 65], bf16_, kind="ExternalOutput").ap()
        io["dbg_x0"] = nc.dram_tensor("dbg_x0", [64, 2048], bf16_, kind="ExternalOutput").ap()
        io["dbg_x1"] = nc.dram_tensor("dbg_x1", [64, 2048], bf16_, kind="ExternalOutput").ap()

    with tile.TileContext(nc) as tc:
        with ExitStack() as ctx:
            _emit(ctx, tc, io)
    nc.compile()
    _CACHE[key] = nc
    return nc


def make_in_maps(query, key, value, wq, bq, wk, bk, wv, bv, wm, bm):
    fb = lambda a: np.ascontiguousarray(np.asarray(a, dtype=np.float32)).astype(BF)
    f = lambda a: np.ascontiguousarray(np.asarray(a), dtype=np.float32)
    query, key, value = f(query), f(key), f(value)
    wq, wk, wv, wm = f(wq), f(wk), f(wv), f(wm)
    bq = f(bq)

    def chan_pack(w, idx):
        # [256,128] (chan, out) -> [128, 2, 128] -> [128, 256] packed
        wt = w[idx].T.reshape(2, 128, 128).transpose(1, 0, 2).reshape(128, 256)
        return wt

    in_maps = []
    for c in range(8):
        b, pair = divmod(c, 2)
        hs = (2 * pair, 2 * pair + 1)
        idx = np.array([d * H + h for h in hs for d in range(DIM)])
        blob = np.concatenate(
            [
                chan_pack(wq, idx),
                chan_pack(wk, idx),
                chan_pack(wv, idx),
                np.pad(
                    np.concatenate([wm[:, idx[:64]].T, wm[:, idx[64:]].T], axis=1),
                    ((0, 64), (0, 0)),
                ),
            ],
            axis=1,
        )
        m = {
            "xq": fb(query[b]),
            "xk": fb(key[b]),
            "xv": fb(value[b]),
            "wblob": fb(blob),
            "bq": f(bq[idx].reshape(128, 1)),
        }
        in_maps.append(m)
    return in_maps


def run(in_maps, trace=False, **kw):
    from concourse import bass_utils

    nc = _build_nc()
    return bass_utils.run_bass_kernel_spmd(
        nc, in_maps, core_ids=list(range(8)), trace=trace, **kw
    )


def gather(results, wm, bv, bm):
    wm = np.asarray(wm, dtype=np.float32)
    bv = np.asarray(bv, dtype=np.float32)
    bm = np.asarray(bm, dtype=np.float32)
    bias = bm + wm @ bv  # device skips bv; its output contribution is wm @ bv
    outs = [np.asarray(r["out"], dtype=np.float32) for r in results]
    return np.stack([outs[2 * b] + outs[2 * b + 1] + bias[:, None] for b in range(B)])


def kernel(query, key, value, wq, bq, wk, bk, wv, bv, wm, bm):
    in_maps = make_in_maps(query, key, value, wq, bq, wk, bk, wv, bv, wm, bm)
    res = run(in_maps)
    return gather(res.results, wm, bv, bm)
